# revision 1
# baseline (speedup 1.0000x reference)
"""Trainium2 Bass kernel for one GPT-2-style transformer Block.

Reference math: non-causal MHA + tanh-GELU MLP, both pre-LayerNorm with
residual. B=4, T=2048, C=1024, H=16 heads, hd=64.

Strategy: zero-communication data parallelism over 8 NeuronCores.
Core i handles batch b=i//2 and query-token half h=i%2 (1024 tokens).
Each core redundantly computes K,V for its batch's full 2048 tokens
(cheaper than a 2-rank collective), then attention/proj/MLP for its own
1024 query tokens only.

Device layout: activations feature-major [channels, tokens] ("fm"); V is
produced token-major for the attention AV matmul. Host pre-work is pure
layout/algebra: transpose x, fold LN affine into the next matmul
(LN_aff(xhat)@W + b == xhat@(ln_w*W) + (ln_b@W + b)), fold 1/sqrt(hd)
into Wq/bq, cast weights to bf16 (matmuls run bf16, fp32 PSUM accum).

Attention: scores transposed S^T[tk,tq] (two heads of a pair live on
partitions 0-63 / 64-127 and their K=64 matmuls run concurrently via
row-group tiling); exp on ACT into bf16 P^T; AV uses V augmented with a
ones column (M=65) so PSUM row 64 accumulates the softmax denominator
for free; y is normalized with the broadcast reciprocal on copy-back.
LayerNorm channel-stats are ones-vector matmuls on the TensorEngine.
K and V are computed per-head-pair inside the attention loop so their
GEMMs overlap the previous pair's softmax exp on ACT.
"""

import sys

import numpy as np
import ml_dtypes

if "/opt/trn_rl_repo" not in sys.path:
    sys.path.insert(0, "/opt/trn_rl_repo")

P = 128
C = 1024
CT = C // P            # 8 channel tiles
TKV = 2048
TQ = 1024
H = 16
HD = 64
F = 4096
FT = F // P            # 32
NCORES = 8
EPS = 1e-5

_BF16 = ml_dtypes.bfloat16
_CACHE: dict = {}
PHASE_MARKS: list = []


def _build_nc(loop_n: int = 0):
    import concourse.tile as tile
    from concourse import bacc, mybir

    DT_BF = mybir.dt.bfloat16
    DT_F32 = mybir.dt.float32
    AF = mybir.ActivationFunctionType
    OP = mybir.AluOpType

    nc = bacc.Bacc("TRN2", target_bir_lowering=False)

    d_xkv = nc.declare_dram_parameter("xkv_bf", [CT, P, TKV], DT_BF, isOutput=False)
    d_xq = nc.declare_dram_parameter("xq_bf", [CT, P, TQ], DT_BF, isOutput=False)
    d_wqkv = nc.declare_dram_parameter("wqkv", [CT, P, 3 * C], DT_BF, isOutput=False)
    d_bq = nc.declare_dram_parameter("bq", [P, CT], DT_F32, isOutput=False)
    d_bk = nc.declare_dram_parameter("bk", [P, CT], DT_F32, isOutput=False)
    d_bv = nc.declare_dram_parameter("bv", [1, C], DT_BF, isOutput=False)
    d_wproj = nc.declare_dram_parameter("wproj", [CT, P, C], DT_BF, isOutput=False)
    d_bproj = nc.declare_dram_parameter("bproj", [P, CT], DT_F32, isOutput=False)
    d_wfc = nc.declare_dram_parameter("wfc", [CT, P, F], DT_BF, isOutput=False)
    d_bfc = nc.declare_dram_parameter("bfc", [P, FT], DT_F32, isOutput=False)
    d_wfc2 = nc.declare_dram_parameter("wfc2", [CT, P, FT, P], DT_BF, isOutput=False)
    d_bfc2 = nc.declare_dram_parameter("bfc2", [P, CT], DT_F32, isOutput=False)
    d_out = nc.declare_dram_parameter("out", [CT, P, TQ], DT_F32, isOutput=True)

    import contextlib

    with tile.TileContext(nc) as tc:
        loop_ctx = tc.For_i(0, loop_n, 1) if loop_n else contextlib.nullcontext()
        pools = []

        def pool(name, bufs, space="SBUF"):
            pm = tc.tile_pool(name=name, bufs=bufs, space=space)
            pools.append(pm)
            return pm.__enter__()

        singles = pool("singles", 1)
        big = pool("big", 1)          # persistent tensors, explicit tag reuse
        stat = pool("stat", 1)        # LN stat rows (slot-shared across LNs)
        tmp = pool("tmp", 2)          # x^2 chunks
        small = pool("small", 2)      # reciprocal rows / broadcasts / ytmp
        wpool = pool("wpool", 3)      # streamed weight chunks (4KB slots)
        kvw = pool("kvw", 2)          # small per-head-pair weight chunks
        ktp = pool("ktp", 2)          # per-head-pair K tiles
        vtp = pool("vtp", 2)          # per-head-pair V tiles
        ppool = pool("ppool", 2)      # P^T tiles; also reused for h3 halves
        opool = pool("opool", 1)
        ps1 = pool("ps1", 4, space="PSUM")   # 1-bank psums, tag "g"
        ps2 = pool("ps2", 2, space="PSUM")   # 2-bank score psums, tag "sc"

        loop_ctx.__enter__()

        PHASE_MARKS.clear()

        def mark(name):
            PHASE_MARKS.append((name, nc.next_id()))

        mark("setup")
        # constants / biases
        ones_bf = singles.tile([P, 1], DT_BF)
        nc.vector.memset(ones_bf, 1.0)
        eps1 = singles.tile([1, 1], DT_F32)
        nc.vector.memset(eps1, EPS)
        bq_sb = singles.tile([P, CT], DT_F32)
        nc.sync.dma_start(out=bq_sb, in_=d_bq[:, :])
        bk_sb = singles.tile([P, CT], DT_F32)
        nc.sync.dma_start(out=bk_sb, in_=d_bk[:, :])
        bproj_sb = singles.tile([P, CT], DT_F32)
        nc.sync.dma_start(out=bproj_sb, in_=d_bproj[:, :])
        bfc_sb = singles.tile([P, FT], DT_F32)
        nc.sync.dma_start(out=bfc_sb, in_=d_bfc[:, :])
        bfc2_sb = singles.tile([P, CT], DT_F32)
        nc.sync.dma_start(out=bfc2_sb, in_=d_bfc2[:, :])
        bv_row = singles.tile([1, C], DT_BF)
        nc.sync.dma_start(out=bv_row, in_=d_bv[:, :])
        bv_b = singles.tile([P, H, HD], DT_BF)
        nc.gpsimd.partition_broadcast(bv_b[:], bv_row[:])

        def ln_stats(x_bf, ntok):
            """x_bf: [P, CT, ntok] bf16 fm. Returns (mu_b, rstd_b) bf16
            [P, ntok] partition-broadcast tiles (slot-shared across calls)."""
            mubf = stat.tile([1, TKV], DT_BF, tag="mubf")
            rstdbf = stat.tile([1, TKV], DT_BF, tag="rstdbf")
            for tt in range(ntok // 512):
                ts_ = slice(tt * 512, (tt + 1) * 512)
                ps_s = ps1.tile([1, 512], DT_F32, tag="g")
                ps_q = ps1.tile([1, 512], DT_F32, tag="g")
                for ct in range(CT):
                    x2c = tmp.tile([P, 512], DT_BF, tag="x2c")
                    nc.gpsimd.tensor_mul(x2c[:], x_bf[:, ct, ts_], x_bf[:, ct, ts_])
                    nc.tensor.matmul(
                        ps_s[:], ones_bf[:], x_bf[:, ct, ts_],
                        start=(ct == 0), stop=(ct == CT - 1))
                    nc.tensor.matmul(
                        ps_q[:], ones_bf[:], x2c[:],
                        start=(ct == 0), stop=(ct == CT - 1))
                nc.vector.tensor_scalar_mul(mubf[:, ts_], ps_s[:], 1.0 / C)
                t1 = stat.tile([1, 512], DT_F32, tag="t1")
                nc.vector.tensor_mul(t1[:], mubf[:, ts_], mubf[:, ts_])
                t2 = stat.tile([1, 512], DT_F32, tag="t2")
                nc.vector.scalar_tensor_tensor(
                    t2[:], ps_q[:], 1.0 / C, t1[:],
                    op0=OP.mult, op1=OP.subtract)
                nc.scalar.activation(out=t1[:], in_=t2[:], func=AF.Sqrt,
                                     bias=eps1[:])
                with nc.allow_low_precision(reason="rstd in bf16 is intended"):
                    nc.vector.reciprocal(rstdbf[:, ts_], t1[:])
            mu_b = stat.tile([P, TKV], DT_BF, tag="mu_b")
            nc.gpsimd.partition_broadcast(mu_b[:, :ntok], mubf[:, :ntok])
            rstd_b = stat.tile([P, TKV], DT_BF, tag="rstd_b")
            nc.gpsimd.partition_broadcast(rstd_b[:, :ntok], rstdbf[:, :ntok])
            return mu_b[:, :ntok], rstd_b[:, :ntok]

        def ln_apply(dst, x_bf, mu_b, rstd_b, ntok):
            nc.vector.tensor_sub(
                dst[:], x_bf[:], mu_b[:, None, :].broadcast_to((P, CT, ntok)))
            nc.vector.tensor_mul(
                dst[:], dst[:], rstd_b[:, None, :].broadcast_to((P, CT, ntok)))

        # ---------- LN1 over Q tokens (Xq kept raw for the residual) ----------
        mark("ln1q")
        Xq = big.tile([P, CT, TQ], DT_BF, tag="Xq")
        for ct in range(CT):
            nc.sync.dma_start(out=Xq[:, ct, :], in_=d_xq.ap()[ct])
        muq_b, rstdq_b = ln_stats(Xq, TQ)
        xcq = big.tile([P, CT, TQ], DT_BF, tag="xcq")
        ln_apply(xcq, Xq, muq_b, rstdq_b, TQ)

        # ---------- LN1 over KV tokens (xc in place over X) ----------
        mark("ln1kv")
        X = big.tile([P, CT, TKV], DT_BF, tag="X")
        for ct in range(CT):
            nc.sync.dma_start(out=X[:, ct, :], in_=d_xkv.ap()[ct])
        mu_b, rstd_b = ln_stats(X, TKV)
        ln_apply(X, X, mu_b, rstd_b, TKV)
        xc = X  # normalized, in place

        # ---------- Q projection (feature-major) ----------
        mark("qgemm")
        Q = big.tile([P, CT, TQ], DT_BF, tag="Q")
        for qch in range(4):
            wch = wpool.tile([P, CT, 256], DT_BF, tag="w4")
            nc.sync.dma_start(
                out=wch[:],
                in_=d_wqkv.ap()[:, :, qch * 256 : (qch + 1) * 256].rearrange(
                    "c p f -> p c f"))
            for fsub in range(2):
                fo = qch * 2 + fsub
                for tt in range(TQ // 512):
                    ps = ps1.tile([P, 512], DT_F32, tag="g")
                    for ci in range(CT):
                        nc.tensor.matmul(
                            ps[:],
                            wch[:, ci, fsub * P : (fsub + 1) * P],
                            xcq[:, ci, tt * 512 : (tt + 1) * 512],
                            start=(ci == 0), stop=(ci == CT - 1))
                    nc.vector.tensor_scalar(
                        out=Q[:, fo, tt * 512 : (tt + 1) * 512],
                        in0=ps[:], scalar1=bq_sb[:, fo : fo + 1], scalar2=None,
                        op0=OP.add)

        Y = big.tile([P, CT, TQ], DT_BF, tag="Y")

        # ---------- attention, K/V streamed per head pair ----------
        for hp in range(CT):
            mark(f"attn{hp}" if hp else "attn0_k")
            # K for this pair: [128ch, TKV] fm
            wk = kvw.tile([P, CT, P], DT_BF, tag="wk")
            nc.sync.dma_start(
                out=wk[:],
                in_=d_wqkv.ap()[:, :, C + hp * P : C + (hp + 1) * P].rearrange(
                    "c p f -> p c f"))
            K_hp = ktp.tile([P, TKV], DT_BF, tag="kt")
            for tt in range(TKV // 512):
                ts_ = slice(tt * 512, (tt + 1) * 512)
                ps = ps1.tile([P, 512], DT_F32, tag="g")
                for ci in range(CT):
                    nc.tensor.matmul(
                        ps[:], wk[:, ci, :], xc[:, ci, ts_],
                        start=(ci == 0), stop=(ci == CT - 1))
                nc.vector.tensor_scalar(
                    out=K_hp[:, ts_], in0=ps[:],
                    scalar1=bk_sb[:, hp : hp + 1], scalar2=None, op0=OP.add)

            if hp == 0:
                mark("attn0_v")
            # V for this pair: token-major [tk, 2, hd+1] with ones column
            wv = kvw.tile([P, CT, P], DT_BF, tag="wv")
            nc.sync.dma_start(
                out=wv[:],
                in_=d_wqkv.ap()[:, :, 2 * C + hp * P : 2 * C + (hp + 1) * P
                                ].rearrange("c p f -> p c f"))
            V_hp = vtp.tile([P, TKV // P, 2, HD + 1], DT_BF, tag="vt")
            nc.vector.memset(V_hp[:, :, :, HD : HD + 1], 1.0)
            for tk in range(TKV // P):
                ps = ps1.tile([P, 512], DT_F32, tag="g")
                for ci in range(CT):
                    nc.tensor.matmul(
                        ps[:, :P],
                        xc[:, ci, tk * P : (tk + 1) * P],
                        wv[:, ci, :],
                        start=(ci == 0), stop=(ci == CT - 1))
                nc.vector.tensor_add(
                    out=V_hp[:, tk, :, 0:HD],
                    in0=ps[:, :P].rearrange("p (h d) -> p h d", h=2),
                    in1=bv_b[:, 2 * hp : 2 * hp + 2, :])

            if hp == 0:
                mark("attn0_sc")
            for tcn in range(TQ // 512):
                tqs = slice(tcn * 512, (tcn + 1) * 512)
                pts = [ppool.tile([P, TKV // P, 512], DT_BF, tag="pt",
                                  name=f"pt{i}") for i in range(2)]
                for g in range(TKV // 256):
                    psc = [ps2.tile([P, 1024], DT_F32, tag="sc",
                                    name=f"sc{i}") for i in range(2)]
                    for k2 in range(2):
                        tk = g * 2 + k2
                        for hi in range(2):
                            bp = hi * 64
                            nc.tensor.matmul(
                                psc[hi][:, k2 * 512 : (k2 + 1) * 512],
                                K_hp[bp : bp + 64, tk * P : (tk + 1) * P],
                                Q[bp : bp + 64, hp, tqs],
                                start=True, stop=True,
                                tile_position=(bp, 0))
                    for hi in range(2):
                        nc.scalar.activation(
                            out=pts[hi][:, g * 2 : g * 2 + 2, :],
                            in_=psc[hi][:].rearrange("p (k t) -> p k t", k=2),
                            func=AF.Exp)
                if hp == 0:
                    mark(f"attn0_av{tcn}")
                for hi in range(2):
                    ps_y = ps1.tile([P, 512], DT_F32, tag="g")
                    for tk in range(TKV // P):
                        nc.tensor.matmul(
                            ps_y[0 : HD + 1, :],
                            V_hp[:, tk, hi, :],
                            pts[hi][:, tk, :],
                            start=(tk == 0), stop=(tk == TKV // P - 1))
                    rrow = small.tile([1, 512], DT_F32, tag="rrow")
                    nc.vector.reciprocal(rrow[:], ps_y[HD : HD + 1, :])
                    rb = small.tile([HD, 512], DT_F32, tag="rb")
                    nc.gpsimd.partition_broadcast(rb[:], rrow[:])
                    if hi == 0:
                        nc.vector.tensor_mul(
                            Y[0:HD, hp, tqs], ps_y[0:HD, :], rb[:])
                    else:
                        # DVE lanes are partition-locked; odd head's rows
                        # must move to partitions 64-127 via DMA.
                        ytmp = small.tile([HD, 512], DT_BF, tag="ytmp")
                        nc.vector.tensor_mul(ytmp[:], ps_y[0:HD, :], rb[:])
                        nc.sync.dma_start(out=Y[HD:P, hp, tqs], in_=ytmp[:])

        # ---------- proj+residual -> LN2 (both chunks), then MLP ----------
        mark("proj")
        x2 = big.tile([P, CT, TQ], DT_BF, tag="xcq")  # reuse xcq slot
        xc2 = big.tile([P, CT, TQ], DT_BF, tag="Q")   # reuse Q slot
        for tcn in range(TQ // 512):
            tqs = slice(tcn * 512, (tcn + 1) * 512)
            if tcn:
                mark(f"proj{tcn}")
            for pch in range(4):
                wch = wpool.tile([P, CT, 256], DT_BF, tag="w4")
                nc.sync.dma_start(
                    out=wch[:],
                    in_=d_wproj.ap()[:, :, pch * 256 : (pch + 1) * 256].rearrange(
                        "c p f -> p c f"))
                for fsub in range(2):
                    co = pch * 2 + fsub
                    ps = ps1.tile([P, 512], DT_F32, tag="g")
                    for ci in range(CT):
                        nc.tensor.matmul(
                            ps[:],
                            wch[:, ci, fsub * P : (fsub + 1) * P],
                            Y[:, ci, tqs],
                            start=(ci == 0), stop=(ci == CT - 1))
                    nc.vector.scalar_tensor_tensor(
                        x2[:, co, tqs], ps[:], bproj_sb[:, co : co + 1],
                        Xq[:, co, tqs], op0=OP.add, op1=OP.add)

            # LN2 for this chunk
            mark(f"ln2_{tcn}")
            mu2_b, rstd2_b = ln_stats(x2[:, :, tqs], 512)
            ln_apply(xc2[:, :, tqs], x2[:, :, tqs], mu2_b, rstd2_b, 512)

        for tcn in range(TQ // 512):
            tqs = slice(tcn * 512, (tcn + 1) * 512)
            # MLP fc + gelu (h3 reuses the P^T slots)
            mark(f"mlp{tcn}")
            h3 = [ppool.tile([P, FT // 2, 512], DT_BF, tag="pt",
                             name=f"h3_{i}") for i in range(2)]
            for fch in range(F // 256):
                wch = wpool.tile([P, CT, 256], DT_BF, tag="w4")
                nc.sync.dma_start(
                    out=wch[:],
                    in_=d_wfc.ap()[:, :, fch * 256 : (fch + 1) * 256].rearrange(
                        "c p f -> p c f"))
                for fsub in range(2):
                    fo = fch * 2 + fsub
                    ps = ps1.tile([P, 512], DT_F32, tag="g")
                    for ci in range(CT):
                        nc.tensor.matmul(
                            ps[:],
                            wch[:, ci, fsub * P : (fsub + 1) * P],
                            xc2[:, ci, tqs],
                            start=(ci == 0), stop=(ci == CT - 1))
                    nc.scalar.activation(
                        out=h3[fo // 16][:, fo % 16, :], in_=ps[:],
                        func=AF.Gelu_apprx_tanh,
                        bias=bfc_sb[:, fo : fo + 1])
            for co in range(CT):
                wc2a = wpool.tile([P, FT // 2, P], DT_BF, tag="w4")
                nc.sync.dma_start(out=wc2a[:], in_=d_wfc2.ap()[co][:, 0 : FT // 2, :])
                wc2b = wpool.tile([P, FT // 2, P], DT_BF, tag="w4")
                nc.sync.dma_start(out=wc2b[:], in_=d_wfc2.ap()[co][:, FT // 2 :, :])
                wparts = (wc2a, wc2b)
                ps = ps1.tile([P, 512], DT_F32, tag="g")
                for fk in range(FT):
                    nc.tensor.matmul(
                        ps[:],
                        wparts[fk // 16][:, fk % 16, :],
                        h3[fk // 16][:, fk % 16, :],
                        start=(fk == 0), stop=(fk == FT - 1))
                osb = opool.tile([P, 512], DT_F32, tag="osb")
                nc.vector.scalar_tensor_tensor(
                    osb[:], ps[:], bfc2_sb[:, co : co + 1], x2[:, co, tqs],
                    op0=OP.add, op1=OP.add)
                nc.sync.dma_start(out=d_out.ap()[co][:, tqs], in_=osb[:])

        loop_ctx.__exit__(None, None, None)

        for pm in reversed(pools):
            pm.__exit__(None, None, None)

    nc.compile()
    return nc


def _get_nc():
    if "nc" not in _CACHE:
        _CACHE["nc"] = _build_nc()
    return _CACHE["nc"]


def _prep_shared(inputs):
    f32 = np.float32
    ln1_w = np.asarray(inputs["ln1_w"], f32)
    ln1_b = np.asarray(inputs["ln1_b"], f32)
    attn_w = np.asarray(inputs["attn_w"], f32)
    attn_b = np.asarray(inputs["attn_b"], f32)
    proj_w = np.asarray(inputs["proj_w"], f32)
    proj_b = np.asarray(inputs["proj_b"], f32)
    ln2_w = np.asarray(inputs["ln2_w"], f32)
    ln2_b = np.asarray(inputs["ln2_b"], f32)
    fc_w = np.asarray(inputs["fc_w"], f32)
    fc_b = np.asarray(inputs["fc_b"], f32)
    fc2_w = np.asarray(inputs["fc2_w"], f32)
    fc2_b = np.asarray(inputs["fc2_b"], f32)

    w1 = (ln1_w[:, None] * attn_w).copy()
    b1 = (ln1_b @ attn_w + attn_b).copy()
    w1[:, :C] *= 0.125          # fold 1/sqrt(hd) into Wq / bq
    b1[:C] *= 0.125
    w2 = ln2_w[:, None] * fc_w
    b2 = ln2_b @ fc_w + fc_b

    return {
        "wqkv": np.ascontiguousarray(w1.reshape(CT, P, 3 * C)).astype(_BF16),
        "bq": np.ascontiguousarray(b1[:C].reshape(CT, P).T).astype(f32),
        "bk": np.ascontiguousarray(b1[C : 2 * C].reshape(CT, P).T).astype(f32),
        "bv": b1[2 * C :].reshape(1, C).astype(_BF16),
        "wproj": np.ascontiguousarray(proj_w.reshape(CT, P, C)).astype(_BF16),
        "bproj": np.ascontiguousarray(proj_b.reshape(CT, P).T).astype(f32),
        "wfc": np.ascontiguousarray(w2.reshape(CT, P, F)).astype(_BF16),
        "bfc": np.ascontiguousarray(b2.reshape(FT, P).T).astype(f32),
        "wfc2": np.ascontiguousarray(
            fc2_w.reshape(FT, P, CT, P).transpose(2, 1, 0, 3)).astype(_BF16),
        "bfc2": np.ascontiguousarray(fc2_b.reshape(CT, P).T).astype(f32),
    }


def _make_in_maps(inputs):
    x = np.asarray(inputs["x"], np.float32)  # [B, T, C]
    shared = _prep_shared(inputs)
    in_maps = []
    for core in range(NCORES):
        b, h = core // 2, core % 2
        xT = np.ascontiguousarray(x[b].T)                           # [C, TKV]
        xqT = np.ascontiguousarray(x[b, h * TQ : (h + 1) * TQ].T)   # [C, TQ]
        m = dict(shared)
        m["xkv_bf"] = xT.reshape(CT, P, TKV).astype(_BF16)
        m["xq_bf"] = xqT.reshape(CT, P, TQ).astype(_BF16)
        in_maps.append(m)
    return in_maps


def kernel(**inputs) -> np.ndarray:
    from concourse.bass_utils import run_bass_kernel_spmd

    nc = _get_nc()
    in_maps = _make_in_maps(inputs)
    res = run_bass_kernel_spmd(nc, in_maps, core_ids=list(range(NCORES)))

    out = np.empty((4, 2048, C), np.float32)
    for core in range(NCORES):
        b, h = core // 2, core % 2
        o = np.asarray(res.results[core]["out"])  # [CT, P, TQ]
        out[b, h * TQ : (h + 1) * TQ, :] = o.reshape(C, TQ).T
    return out



# revision 4
# speedup vs baseline: 1.4733x; 1.4733x over previous
"""Trainium2 Bass kernel for one GPT-2-style transformer Block.

Reference math: non-causal MHA + tanh-GELU MLP, both pre-LayerNorm with
residual. B=4, T=2048, C=1024, H=16 heads, hd=64.

Strategy: zero-communication data parallelism over 8 NeuronCores.
Core i handles batch b=i//2 and query-token half h=i%2 (1024 tokens).
Each core redundantly computes K,V for its batch's full 2048 tokens
(cheaper than a 2-rank collective), then attention/proj/MLP for its own
1024 query tokens only. The kv token order is host-permuted so the
core's own query tokens are always columns [0, 1024) — softmax over kv
is order-invariant, and this lets every core address its q slice at a
fixed offset (no per-core xq copy, single LN1 pass).

Device layout: activations feature-major [channels, tokens] ("fm"); V is
produced token-major for the attention AV matmul. Host pre-work is pure
layout/algebra: transpose x, fold LN affine into the next matmul
(LN_aff(xhat)@W + b == xhat@(ln_w*W) + (ln_b@W + b)), fold 1/sqrt(hd)
into Wq/bq.

fp8: QKV/V/AV/proj GEMMs run float8e4 with MatmulPerfMode.DoubleRow
(two 128-deep k-subtiles per instruction -> 2x PE throughput). Weights
are host-scaled by SW=32 (SQ=256 for the 0.125-folded Wq) into fp8
range; the inverse scale rides the PSUM->SBUF bias-add for free.
LN1 output xc is written fp8; exp output P^T is written fp8 by the
activation op; Y is fp8 into the proj GEMM. Scores stay bf16 (the
64-deep per-head contraction cannot use DoubleRow without waste) and
the MLP stays bf16 (fp8 there costs ~1.6e-2 max-rel: too close to the
2e-2 gate).

Attention: scores transposed S^T[tk,tq] (two heads of a pair live on
partitions 0-63 / 64-127 and their K=64 matmuls run concurrently via
row-group tiling); exp on ACT into fp8 P^T; AV uses V augmented with a
ones column (M=65) so PSUM row 64 accumulates the softmax denominator
for free; y is normalized with the broadcast reciprocal on copy-back.
LayerNorm channel-stats are ones-vector matmuls on the TensorEngine.
K and V are computed per-head-pair inside the attention loop so their
GEMMs overlap the previous pair's softmax exp on ACT.
"""

import sys

import numpy as np
import ml_dtypes

if "/opt/trn_rl_repo" not in sys.path:
    sys.path.insert(0, "/opt/trn_rl_repo")

P = 128
C = 1024
CT = C // P            # 8 channel tiles
TKV = 2048
TQ = 1024
H = 16
HD = 64
F = 4096
FT = F // P            # 32
NCORES = 8
EPS = 1e-5
SW = 32.0              # fp8 weight scale (K/V/proj)
SQ = 256.0             # fp8 weight scale for Wq (0.125 pre-folded)

_BF16 = ml_dtypes.bfloat16
_F8 = ml_dtypes.float8_e4m3
_CACHE: dict = {}
PHASE_MARKS: list = []


def _build_nc(loop_n: int = 0):
    import concourse.tile as tile
    from concourse import bacc, mybir

    DT_BF = mybir.dt.bfloat16
    DT_F8 = mybir.dt.float8e4
    DT_F32 = mybir.dt.float32
    AF = mybir.ActivationFunctionType
    OP = mybir.AluOpType
    PM = mybir.MatmulPerfMode

    nc = bacc.Bacc("TRN2", target_bir_lowering=False)

    d_xkv = nc.declare_dram_parameter("xkv_bf", [CT, P, TKV], DT_BF, isOutput=False)
    d_wqkv = nc.declare_dram_parameter("wqkv", [CT, P, 3 * C], DT_F8, isOutput=False)
    d_bq = nc.declare_dram_parameter("bq", [P, CT], DT_F32, isOutput=False)
    d_bk = nc.declare_dram_parameter("bk", [P, CT], DT_F32, isOutput=False)
    d_bv = nc.declare_dram_parameter("bv", [1, C], DT_BF, isOutput=False)
    d_wproj = nc.declare_dram_parameter("wproj", [CT, P, C], DT_F8, isOutput=False)
    d_bproj = nc.declare_dram_parameter("bproj", [P, CT], DT_F32, isOutput=False)
    d_wfc = nc.declare_dram_parameter("wfc", [CT, P, F], DT_BF, isOutput=False)
    d_bfc = nc.declare_dram_parameter("bfc", [P, FT], DT_F32, isOutput=False)
    d_wfc2 = nc.declare_dram_parameter("wfc2", [CT, P, FT, P], DT_BF, isOutput=False)
    d_bfc2 = nc.declare_dram_parameter("bfc2", [P, CT], DT_F32, isOutput=False)
    d_out = nc.declare_dram_parameter("out", [CT, P, TQ], DT_F32, isOutput=True)

    import contextlib

    with tile.TileContext(nc) as tc:
        loop_ctx = tc.For_i(0, loop_n, 1) if loop_n else contextlib.nullcontext()
        pools = []

        def pool(name, bufs, space="SBUF"):
            pm = tc.tile_pool(name=name, bufs=bufs, space=space)
            pools.append(pm)
            return pm.__enter__()

        singles = pool("singles", 1)
        big = pool("big", 1)          # persistent tensors, explicit tag reuse
        stat = pool("stat", 1)        # LN stat rows (slot-shared across LNs)
        tmp = pool("tmp", 2)          # x^2 chunks
        small = pool("small", 2)      # reciprocal rows / broadcasts / ytmp
        wpool = pool("wpool", 3)      # streamed weight chunks
        kvw = pool("kvw", 2)          # small per-head-pair weight chunks
        ktp = pool("ktp", 2)          # per-head-pair K tiles
        vtp = pool("vtp", 2)          # per-head-pair V tiles
        ppool = pool("ppool", 2)      # P^T tiles (fp8) + h3 halves (bf16)
        opool = pool("opool", 1)
        ps1 = pool("ps1", 4, space="PSUM")   # 1-bank psums, tag "g"
        ps2 = pool("ps2", 2, space="PSUM")   # 2-bank score psums, tag "sc"

        loop_ctx.__enter__()

        PHASE_MARKS.clear()

        def mark(name):
            PHASE_MARKS.append((name, nc.next_id()))

        mark("setup")
        # constants / biases
        ones_bf = singles.tile([P, 1], DT_BF)
        nc.vector.memset(ones_bf, 1.0)
        eps1 = singles.tile([1, 1], DT_F32)
        nc.vector.memset(eps1, EPS)
        bq_sb = singles.tile([P, CT], DT_F32)
        nc.sync.dma_start(out=bq_sb, in_=d_bq[:, :])
        bk_sb = singles.tile([P, CT], DT_F32)
        nc.sync.dma_start(out=bk_sb, in_=d_bk[:, :])
        bproj_sb = singles.tile([P, CT], DT_F32)
        nc.sync.dma_start(out=bproj_sb, in_=d_bproj[:, :])
        bfc_sb = singles.tile([P, FT], DT_F32)
        nc.sync.dma_start(out=bfc_sb, in_=d_bfc[:, :])
        bfc2_sb = singles.tile([P, CT], DT_F32)
        nc.sync.dma_start(out=bfc2_sb, in_=d_bfc2[:, :])
        bv_row = singles.tile([1, C], DT_BF)
        nc.sync.dma_start(out=bv_row, in_=d_bv[:, :])
        bv_b = singles.tile([P, H, HD], DT_BF)
        nc.gpsimd.partition_broadcast(bv_b[:], bv_row[:])

        def ln_stats(x_bf, ntok):
            """x_bf: [P, CT, ntok] bf16 fm. Returns (mu_b, rstd_b) bf16
            [P, ntok] partition-broadcast tiles (slot-shared across calls)."""
            mubf = stat.tile([1, TKV], DT_BF, tag="mubf")
            rstdbf = stat.tile([1, TKV], DT_BF, tag="rstdbf")
            for tt in range(ntok // 512):
                ts_ = slice(tt * 512, (tt + 1) * 512)
                ps_s = ps1.tile([1, 512], DT_F32, tag="g")
                ps_q = ps1.tile([1, 512], DT_F32, tag="g")
                for ct in range(CT):
                    x2c = tmp.tile([P, 512], DT_BF, tag="x2c")
                    nc.gpsimd.tensor_mul(x2c[:], x_bf[:, ct, ts_], x_bf[:, ct, ts_])
                    nc.tensor.matmul(
                        ps_s[:], ones_bf[:], x_bf[:, ct, ts_],
                        start=(ct == 0), stop=(ct == CT - 1))
                    nc.tensor.matmul(
                        ps_q[:], ones_bf[:], x2c[:],
                        start=(ct == 0), stop=(ct == CT - 1))
                nc.vector.tensor_scalar_mul(mubf[:, ts_], ps_s[:], 1.0 / C)
                t1 = stat.tile([1, 512], DT_F32, tag="t1")
                nc.vector.tensor_mul(t1[:], mubf[:, ts_], mubf[:, ts_])
                t2 = stat.tile([1, 512], DT_F32, tag="t2")
                nc.vector.scalar_tensor_tensor(
                    t2[:], ps_q[:], 1.0 / C, t1[:],
                    op0=OP.mult, op1=OP.subtract)
                nc.scalar.activation(out=t1[:], in_=t2[:], func=AF.Sqrt,
                                     bias=eps1[:])
                with nc.allow_low_precision(reason="rstd in bf16 is intended"):
                    nc.vector.reciprocal(rstdbf[:, ts_], t1[:])
            mu_b = stat.tile([P, TKV], DT_BF, tag="mu_b")
            nc.gpsimd.partition_broadcast(mu_b[:, :ntok], mubf[:, :ntok])
            rstd_b = stat.tile([P, TKV], DT_BF, tag="rstd_b")
            nc.gpsimd.partition_broadcast(rstd_b[:, :ntok], rstdbf[:, :ntok])
            return mu_b[:, :ntok], rstd_b[:, :ntok]

        def ln_apply(dst, x_bf, mu_b, rstd_b, ntok):
            nc.vector.tensor_sub(
                dst[:], x_bf[:], mu_b[:, None, :].broadcast_to((P, CT, ntok)))
            nc.vector.tensor_mul(
                dst[:], dst[:], rstd_b[:, None, :].broadcast_to((P, CT, ntok)))

        # ---------- LN1 over all kv tokens (q tokens are cols [0,TQ)) ----
        mark("ln1kv")
        X = big.tile([P, CT, TKV], DT_BF, tag="X")
        for ct in range(CT):
            nc.sync.dma_start(out=X[:, ct, :], in_=d_xkv.ap()[ct])
        mu_b, rstd_b = ln_stats(X, TKV)
        xc = big.tile([P, CT, TKV], DT_F8, tag="xc")
        ln_apply(xc, X, mu_b, rstd_b, TKV)
        # X raw stays for the attention residual; pre-add the proj bias so
        # the proj copy-back is a single scalar_tensor_tensor.
        for ct in range(CT):
            nc.vector.tensor_scalar(
                out=X[:, ct, :TQ], in0=X[:, ct, :TQ],
                scalar1=bproj_sb[:, ct : ct + 1], scalar2=None, op0=OP.add)

        # ---------- Q projection (feature-major, fp8 DoubleRow) ----------
        mark("qgemm")
        Q = big.tile([P, CT, TQ], DT_BF, tag="Q")
        for qch in range(4):
            wch = wpool.tile([P, CT, 256], DT_F8, tag="w8")
            nc.sync.dma_start(
                out=wch[:],
                in_=d_wqkv.ap()[:, :, qch * 256 : (qch + 1) * 256].rearrange(
                    "c p f -> p c f"))
            for fsub in range(2):
                fo = qch * 2 + fsub
                for tt in range(TQ // 512):
                    ps = ps1.tile([P, 512], DT_F32, tag="g")
                    for cp in range(CT // 2):
                        nc.tensor.matmul(
                            ps[:],
                            wch[:, 2 * cp : 2 * cp + 2, fsub * P : (fsub + 1) * P],
                            xc[:, 2 * cp : 2 * cp + 2, tt * 512 : (tt + 1) * 512],
                            start=(cp == 0), stop=(cp == CT // 2 - 1),
                            perf_mode=PM.DoubleRow)
                    nc.vector.tensor_scalar(
                        out=Q[:, fo, tt * 512 : (tt + 1) * 512],
                        in0=ps[:], scalar1=1.0 / SQ,
                        scalar2=bq_sb[:, fo : fo + 1],
                        op0=OP.mult, op1=OP.add)

        Y = big.tile([P, CT, TQ], DT_F8, tag="Y")

        # ---------- attention, K/V streamed per head pair ----------
        for hp in range(CT):
            mark(f"attn{hp}" if hp else "attn0_k")
            # K for this pair: [128ch, TKV] fm bf16 (scores stay bf16)
            wk = kvw.tile([P, CT, P], DT_F8, tag="wk")
            nc.sync.dma_start(
                out=wk[:],
                in_=d_wqkv.ap()[:, :, C + hp * P : C + (hp + 1) * P].rearrange(
                    "c p f -> p c f"))
            K_hp = ktp.tile([P, TKV], DT_BF, tag="kt")
            for tt in range(TKV // 512):
                ts_ = slice(tt * 512, (tt + 1) * 512)
                ps = ps1.tile([P, 512], DT_F32, tag="g")
                for cp in range(CT // 2):
                    nc.tensor.matmul(
                        ps[:], wk[:, 2 * cp : 2 * cp + 2, :],
                        xc[:, 2 * cp : 2 * cp + 2, ts_],
                        start=(cp == 0), stop=(cp == CT // 2 - 1),
                        perf_mode=PM.DoubleRow)
                nc.vector.tensor_scalar(
                    out=K_hp[:, ts_], in0=ps[:], scalar1=1.0 / SW,
                    scalar2=bk_sb[:, hp : hp + 1], op0=OP.mult, op1=OP.add)

            if hp == 0:
                mark("attn0_v")
            # V for this pair: token-major [tk, 2, hd+1] fp8 with ones col
            wv = kvw.tile([P, CT, P], DT_F8, tag="wv")
            nc.sync.dma_start(
                out=wv[:],
                in_=d_wqkv.ap()[:, :, 2 * C + hp * P : 2 * C + (hp + 1) * P
                                ].rearrange("c p f -> p c f"))
            V_hp = vtp.tile([P, 2, TKV // (2 * P), 2, HD + 2], DT_F8, tag="vt")
            nc.vector.memset(V_hp[:, :, :, :, HD : HD + 2], 1.0)
            for tk in range(TKV // P):
                ps = ps1.tile([P, 512], DT_F32, tag="g")
                for cp in range(CT // 2):
                    nc.tensor.matmul(
                        ps[:, :P],
                        xc[:, 2 * cp : 2 * cp + 2, tk * P : (tk + 1) * P],
                        wv[:, 2 * cp : 2 * cp + 2, :],
                        start=(cp == 0), stop=(cp == CT // 2 - 1),
                        perf_mode=PM.DoubleRow)
                nc.vector.scalar_tensor_tensor(
                    V_hp[:, tk % 2, tk // 2, :, 0:HD],
                    ps[:, :P].rearrange("p (h d) -> p h d", h=2), 1.0 / SW,
                    bv_b[:, 2 * hp : 2 * hp + 2, :],
                    op0=OP.mult, op1=OP.add)

            if hp == 0:
                mark("attn0_sc")
            for tcn in range(TQ // 512):
                tqs = slice(tcn * 512, (tcn + 1) * 512)
                pts = [ppool.tile([P, TKV // P, 512], DT_F8, tag="pt8",
                                  name=f"pt{i}") for i in range(2)]
                for g in range(TKV // 256):
                    psc = [ps2.tile([P, 1024], DT_F32, tag="sc",
                                    name=f"sc{i}") for i in range(2)]
                    for k2 in range(2):
                        tk = g * 2 + k2
                        for hi in range(2):
                            bp = hi * 64
                            nc.tensor.matmul(
                                psc[hi][:, k2 * 512 : (k2 + 1) * 512],
                                K_hp[bp : bp + 64, tk * P : (tk + 1) * P],
                                Q[bp : bp + 64, hp, tqs],
                                start=True, stop=True,
                                tile_position=(bp, 0))
                    for hi in range(2):
                        nc.scalar.activation(
                            out=pts[hi][:, g * 2 : g * 2 + 2, :],
                            in_=psc[hi][:].rearrange("p (k t) -> p k t", k=2),
                            func=AF.Exp)
                if hp == 0:
                    mark(f"attn0_av{tcn}")
                for hi in range(2):
                    ps_y = ps1.tile([P, 512], DT_F32, tag="g")
                    for g in range(TKV // 256):
                        nc.tensor.matmul(
                            ps_y[0 : HD + 2, :],
                            V_hp[:, :, g, hi, :],
                            pts[hi][:, 2 * g : 2 * g + 2, :],
                            start=(g == 0), stop=(g == TKV // 256 - 1),
                            perf_mode=PM.DoubleRow)
                    rrow = small.tile([1, 512], DT_F32, tag="rrow")
                    nc.vector.reciprocal(rrow[:], ps_y[HD : HD + 1, :])
                    rb = small.tile([HD, 512], DT_F32, tag="rb")
                    nc.gpsimd.partition_broadcast(rb[:], rrow[:])
                    if hi == 0:
                        nc.vector.tensor_mul(
                            Y[0:HD, hp, tqs], ps_y[0:HD, :], rb[:])
                    else:
                        # DVE lanes are partition-locked; odd head's rows
                        # must move to partitions 64-127 via DMA.
                        ytmp = small.tile([HD, 512], DT_F8, tag="ytmp")
                        nc.vector.tensor_mul(ytmp[:], ps_y[0:HD, :], rb[:])
                        nc.sync.dma_start(out=Y[HD:P, hp, tqs], in_=ytmp[:])

        # ---------- proj+residual -> LN2 (both chunks), then MLP ----------
        mark("proj")
        x2 = big.tile([P, CT, TQ], DT_BF, tag="x2")
        xc2 = big.tile([P, CT, TQ], DT_BF, tag="Q")   # reuse Q slot
        for tcn in range(TQ // 512):
            tqs = slice(tcn * 512, (tcn + 1) * 512)
            if tcn:
                mark(f"proj{tcn}")
            for pch in range(4):
                wch = wpool.tile([P, CT, 256], DT_F8, tag="w8")
                nc.sync.dma_start(
                    out=wch[:],
                    in_=d_wproj.ap()[:, :, pch * 256 : (pch + 1) * 256].rearrange(
                        "c p f -> p c f"))
                for fsub in range(2):
                    co = pch * 2 + fsub
                    ps = ps1.tile([P, 512], DT_F32, tag="g")
                    for cp in range(CT // 2):
                        nc.tensor.matmul(
                            ps[:],
                            wch[:, 2 * cp : 2 * cp + 2, fsub * P : (fsub + 1) * P],
                            Y[:, 2 * cp : 2 * cp + 2, tqs],
                            start=(cp == 0), stop=(cp == CT // 2 - 1),
                            perf_mode=PM.DoubleRow)
                    # x2 = ps/SW + (Xq + bproj)   (bias pre-added into X)
                    nc.vector.scalar_tensor_tensor(
                        x2[:, co, tqs], ps[:], 1.0 / SW,
                        X[:, co, tqs], op0=OP.mult, op1=OP.add)

            # LN2 for this chunk
            mark(f"ln2_{tcn}")
            mu2_b, rstd2_b = ln_stats(x2[:, :, tqs], 512)
            ln_apply(xc2[:, :, tqs], x2[:, :, tqs], mu2_b, rstd2_b, 512)

        for tcn in range(TQ // 512):
            tqs = slice(tcn * 512, (tcn + 1) * 512)
            # MLP fc + gelu (bf16)
            mark(f"mlp{tcn}")
            h3 = [ppool.tile([P, FT // 2, 512], DT_BF, tag="pt",
                             name=f"h3_{i}") for i in range(2)]
            for fch in range(F // 256):
                wch = wpool.tile([P, CT, 256], DT_BF, tag="w4")
                nc.sync.dma_start(
                    out=wch[:],
                    in_=d_wfc.ap()[:, :, fch * 256 : (fch + 1) * 256].rearrange(
                        "c p f -> p c f"))
                for fsub in range(2):
                    fo = fch * 2 + fsub
                    ps = ps1.tile([P, 512], DT_F32, tag="g")
                    for ci in range(CT):
                        nc.tensor.matmul(
                            ps[:],
                            wch[:, ci, fsub * P : (fsub + 1) * P],
                            xc2[:, ci, tqs],
                            start=(ci == 0), stop=(ci == CT - 1))
                    nc.scalar.activation(
                        out=h3[fo // 16][:, fo % 16, :], in_=ps[:],
                        func=AF.Gelu_apprx_tanh,
                        bias=bfc_sb[:, fo : fo + 1])
            for co in range(CT):
                wc2a = wpool.tile([P, FT // 2, P], DT_BF, tag="w4")
                nc.sync.dma_start(out=wc2a[:], in_=d_wfc2.ap()[co][:, 0 : FT // 2, :])
                wc2b = wpool.tile([P, FT // 2, P], DT_BF, tag="w4")
                nc.sync.dma_start(out=wc2b[:], in_=d_wfc2.ap()[co][:, FT // 2 :, :])
                wparts = (wc2a, wc2b)
                ps = ps1.tile([P, 512], DT_F32, tag="g")
                for fk in range(FT):
                    nc.tensor.matmul(
                        ps[:],
                        wparts[fk // 16][:, fk % 16, :],
                        h3[fk // 16][:, fk % 16, :],
                        start=(fk == 0), stop=(fk == FT - 1))
                osb = opool.tile([P, 512], DT_F32, tag="osb")
                nc.vector.scalar_tensor_tensor(
                    osb[:], ps[:], bfc2_sb[:, co : co + 1], x2[:, co, tqs],
                    op0=OP.add, op1=OP.add)
                nc.sync.dma_start(out=d_out.ap()[co][:, tqs], in_=osb[:])

        loop_ctx.__exit__(None, None, None)

        for pm in reversed(pools):
            pm.__exit__(None, None, None)

    nc.compile()
    return nc


def _get_nc():
    if "nc" not in _CACHE:
        _CACHE["nc"] = _build_nc()
    return _CACHE["nc"]


def _prep_shared(inputs):
    f32 = np.float32
    ln1_w = np.asarray(inputs["ln1_w"], f32)
    ln1_b = np.asarray(inputs["ln1_b"], f32)
    attn_w = np.asarray(inputs["attn_w"], f32)
    attn_b = np.asarray(inputs["attn_b"], f32)
    proj_w = np.asarray(inputs["proj_w"], f32)
    proj_b = np.asarray(inputs["proj_b"], f32)
    ln2_w = np.asarray(inputs["ln2_w"], f32)
    ln2_b = np.asarray(inputs["ln2_b"], f32)
    fc_w = np.asarray(inputs["fc_w"], f32)
    fc_b = np.asarray(inputs["fc_b"], f32)
    fc2_w = np.asarray(inputs["fc2_w"], f32)
    fc2_b = np.asarray(inputs["fc2_b"], f32)

    w1 = (ln1_w[:, None] * attn_w).copy()
    b1 = (ln1_b @ attn_w + attn_b).copy()
    w1[:, :C] *= 0.125 * SQ     # fold 1/sqrt(hd) + fp8 scale into Wq
    b1[:C] *= 0.125             # bias applied after the 1/SQ unscale
    w1[:, C:] *= SW             # fp8 scale for Wk/Wv
    w2 = ln2_w[:, None] * fc_w
    b2 = ln2_b @ fc_w + fc_b

    return {
        "wqkv": np.ascontiguousarray(w1.reshape(CT, P, 3 * C)).astype(_F8),
        "bq": np.ascontiguousarray(b1[:C].reshape(CT, P).T).astype(f32),
        "bk": np.ascontiguousarray(b1[C : 2 * C].reshape(CT, P).T).astype(f32),
        "bv": b1[2 * C :].reshape(1, C).astype(_BF16),
        "wproj": np.ascontiguousarray((proj_w * SW).reshape(CT, P, C)).astype(_F8),
        "bproj": np.ascontiguousarray(proj_b.reshape(CT, P).T).astype(f32),
        "wfc": np.ascontiguousarray(w2.reshape(CT, P, F)).astype(_BF16),
        "bfc": np.ascontiguousarray(b2.reshape(FT, P).T).astype(f32),
        "wfc2": np.ascontiguousarray(
            fc2_w.reshape(FT, P, CT, P).transpose(2, 1, 0, 3)).astype(_BF16),
        "bfc2": np.ascontiguousarray(fc2_b.reshape(CT, P).T).astype(f32),
    }


def _make_in_maps(inputs):
    x = np.asarray(inputs["x"], np.float32)  # [B, T, C]
    shared = _prep_shared(inputs)
    in_maps = []
    for core in range(NCORES):
        b, h = core // 2, core % 2
        # permute kv tokens: this core's q half first
        xp = np.concatenate(
            [x[b, h * TQ : (h + 1) * TQ], x[b, (1 - h) * TQ : (2 - h) * TQ]], 0)
        xT = np.ascontiguousarray(xp.T)                             # [C, TKV]
        m = dict(shared)
        m["xkv_bf"] = xT.reshape(CT, P, TKV).astype(_BF16)
        in_maps.append(m)
    return in_maps


def kernel(**inputs) -> np.ndarray:
    from concourse.bass_utils import run_bass_kernel_spmd

    nc = _get_nc()
    in_maps = _make_in_maps(inputs)
    res = run_bass_kernel_spmd(nc, in_maps, core_ids=list(range(NCORES)))

    out = np.empty((4, 2048, C), np.float32)
    for core in range(NCORES):
        b, h = core // 2, core % 2
        o = np.asarray(res.results[core]["out"])  # [CT, P, TQ]
        out[b, h * TQ : (h + 1) * TQ, :] = o.reshape(C, TQ).T
    return out


# revision 5
# speedup vs baseline: 2.1321x; 1.4471x over previous
"""Trainium2 Bass kernel for one GPT-2-style transformer Block.

Reference math: non-causal MHA + tanh-GELU MLP, both pre-LayerNorm with
residual. B=4, T=2048, C=1024, H=16 heads, hd=64.

Strategy: zero-communication data parallelism over 8 NeuronCores.
Core i handles batch b=i//2 and query-token half h=i%2 (1024 tokens).
Each core redundantly computes K,V for its batch's full 2048 tokens
(cheaper than a 2-rank collective), then attention/proj/MLP for its own
1024 query tokens only. The kv token order is host-permuted so the
core's own query tokens are always columns [0, 1024) — softmax over kv
is order-invariant, and this lets every core address its q slice at a
fixed offset (no per-core xq copy, single LN1 pass).

Device layout: activations feature-major [channels, tokens] ("fm"); V is
produced token-major for the attention AV matmul. Host pre-work is pure
layout/algebra: transpose x, fold LN affine into the next matmul
(LN_aff(xhat)@W + b == xhat@(ln_w*W) + (ln_b@W + b)), fold 1/sqrt(hd)
into Wq/bq.

fp8: QKV/V/AV/proj GEMMs run float8e4 with MatmulPerfMode.DoubleRow
(two 128-deep k-subtiles per instruction -> 2x PE throughput). Weights
are host-scaled by SW=32 (SQ=256 for the 0.125-folded Wq) into fp8
range; the inverse scale rides the PSUM->SBUF bias-add for free.
LN1 output xc is written fp8; exp output P^T is written fp8 by the
activation op; Y is fp8 into the proj GEMM. Scores stay bf16 (the
64-deep per-head contraction cannot use DoubleRow without waste) and
the MLP stays bf16 (fp8 there costs ~1.6e-2 max-rel: too close to the
2e-2 gate).

Attention: scores transposed S^T[tk,tq] (two heads of a pair live on
partitions 0-63 / 64-127 and their K=64 matmuls run concurrently via
row-group tiling); exp on ACT into fp8 P^T; AV uses V augmented with
ones columns (M=66, parity-blocked [P,2,8,2,66] so the DoubleRow
k-subtile stride meets the 16-element ISA alignment rule) and PSUM row
64 accumulates the softmax denominator for free; y is normalized with
the broadcast reciprocal on copy-back.
LayerNorm channel-stats are ones-vector matmuls on the TensorEngine.
K and V are computed per-head-pair inside the attention loop so their
GEMMs overlap the previous pair's softmax exp on ACT.
"""

import sys

import numpy as np
import ml_dtypes

if "/opt/trn_rl_repo" not in sys.path:
    sys.path.insert(0, "/opt/trn_rl_repo")

P = 128
C = 1024
CT = C // P            # 8 channel tiles
TKV = 2048
TQ = 1024
H = 16
HD = 64
F = 4096
FT = F // P            # 32
NCORES = 8
EPS = 1e-5
SW = 32.0              # fp8 weight scale (K/V/proj)
SQ = 256.0             # fp8 weight scale for Wq (0.125 pre-folded)

_BF16 = ml_dtypes.bfloat16
_F8 = ml_dtypes.float8_e4m3
_CACHE: dict = {}
PHASE_MARKS: list = []


def _build_nc(loop_n: int = 0):
    import concourse.tile as tile
    from concourse import bacc, mybir

    DT_BF = mybir.dt.bfloat16
    DT_F8 = mybir.dt.float8e4
    DT_F32 = mybir.dt.float32
    AF = mybir.ActivationFunctionType
    OP = mybir.AluOpType
    PM = mybir.MatmulPerfMode

    nc = bacc.Bacc("TRN2", target_bir_lowering=False)

    d_xkv = nc.declare_dram_parameter("xkv_bf", [CT, P, TKV], DT_BF, isOutput=False)
    d_wqkv = nc.declare_dram_parameter("wqkv", [CT, P, 3 * C], DT_F8, isOutput=False)
    d_bq = nc.declare_dram_parameter("bq", [P, CT], DT_F32, isOutput=False)
    d_bk = nc.declare_dram_parameter("bk", [P, CT], DT_F32, isOutput=False)
    d_bv = nc.declare_dram_parameter("bv", [1, C], DT_BF, isOutput=False)
    d_wproj = nc.declare_dram_parameter("wproj", [CT, P, C], DT_F8, isOutput=False)
    d_bproj = nc.declare_dram_parameter("bproj", [P, CT], DT_F32, isOutput=False)
    d_wfc = nc.declare_dram_parameter("wfc", [CT, P, F], DT_BF, isOutput=False)
    d_bfc = nc.declare_dram_parameter("bfc", [P, FT], DT_F32, isOutput=False)
    d_wfc2 = nc.declare_dram_parameter("wfc2", [CT, P, FT, P], DT_BF, isOutput=False)
    d_bfc2 = nc.declare_dram_parameter("bfc2", [P, CT], DT_F32, isOutput=False)
    d_out = nc.declare_dram_parameter("out", [CT, P, TQ], DT_F32, isOutput=True)

    import contextlib

    with tile.TileContext(nc) as tc:
        loop_ctx = tc.For_i(0, loop_n, 1) if loop_n else contextlib.nullcontext()
        pools = []

        def pool(name, bufs, space="SBUF"):
            pm = tc.tile_pool(name=name, bufs=bufs, space=space)
            pools.append(pm)
            return pm.__enter__()

        singles = pool("singles", 1)
        big = pool("big", 1)          # persistent tensors, explicit tag reuse
        stat = pool("stat", 1)        # LN stat rows (slot-shared across LNs)
        tmp = pool("tmp", 2)          # x^2 chunks
        small = pool("small", 2)      # reciprocal rows / broadcasts / ytmp
        wpool = pool("wpool", 3)      # streamed weight chunks
        kvw = pool("kvw", 2)          # small per-head-pair weight chunks
        ktp = pool("ktp", 2)          # per-head-pair K tiles
        vtp = pool("vtp", 2)          # per-head-pair V tiles
        ppool = pool("ppool", 2)      # P^T tiles (fp8) + h3 halves (bf16)
        opool = pool("opool", 1)
        ps1 = pool("ps1", 4, space="PSUM")   # 1-bank psums, tag "g"
        ps2 = pool("ps2", 2, space="PSUM")   # 2-bank score psums, tag "sc"

        loop_ctx.__enter__()

        PHASE_MARKS.clear()

        def mark(name):
            PHASE_MARKS.append((name, nc.next_id()))

        mark("setup")
        # constants / biases
        ones_bf = singles.tile([P, 1], DT_BF)
        nc.vector.memset(ones_bf, 1.0)
        eps1 = singles.tile([1, 1], DT_F32)
        nc.vector.memset(eps1, EPS)
        bq_sb = singles.tile([P, CT], DT_F32)
        nc.sync.dma_start(out=bq_sb, in_=d_bq[:, :])
        bk_sb = singles.tile([P, CT], DT_F32)
        nc.sync.dma_start(out=bk_sb, in_=d_bk[:, :])
        bproj_sb = singles.tile([P, CT], DT_F32)
        nc.sync.dma_start(out=bproj_sb, in_=d_bproj[:, :])
        bfc_sb = singles.tile([P, FT], DT_F32)
        nc.sync.dma_start(out=bfc_sb, in_=d_bfc[:, :])
        bfc2_sb = singles.tile([P, CT], DT_F32)
        nc.sync.dma_start(out=bfc2_sb, in_=d_bfc2[:, :])
        bv_row = singles.tile([1, C], DT_BF)
        nc.sync.dma_start(out=bv_row, in_=d_bv[:, :])
        bv_b = singles.tile([P, H, HD], DT_BF)
        nc.gpsimd.partition_broadcast(bv_b[:], bv_row[:])

        def ln_stats(x_bf, ntok):
            """x_bf: [P, CT, ntok] bf16 fm. Returns (mu_b, rstd_b) bf16
            [P, ntok] partition-broadcast tiles (slot-shared across calls)."""
            mubf = stat.tile([1, TKV], DT_BF, tag="mubf")
            rstdbf = stat.tile([1, TKV], DT_BF, tag="rstdbf")
            for tt in range(ntok // 512):
                ts_ = slice(tt * 512, (tt + 1) * 512)
                ps_s = ps1.tile([1, 512], DT_F32, tag="g")
                ps_q = ps1.tile([1, 512], DT_F32, tag="g")
                for ct in range(CT):
                    x2c = tmp.tile([P, 512], DT_BF, tag="x2c")
                    nc.gpsimd.tensor_mul(x2c[:], x_bf[:, ct, ts_], x_bf[:, ct, ts_])
                    nc.tensor.matmul(
                        ps_s[:], ones_bf[:], x_bf[:, ct, ts_],
                        start=(ct == 0), stop=(ct == CT - 1))
                    nc.tensor.matmul(
                        ps_q[:], ones_bf[:], x2c[:],
                        start=(ct == 0), stop=(ct == CT - 1))
                nc.vector.tensor_scalar_mul(mubf[:, ts_], ps_s[:], 1.0 / C)
                t1 = stat.tile([1, 512], DT_F32, tag="t1")
                nc.vector.tensor_mul(t1[:], mubf[:, ts_], mubf[:, ts_])
                t2 = stat.tile([1, 512], DT_F32, tag="t2")
                nc.vector.scalar_tensor_tensor(
                    t2[:], ps_q[:], 1.0 / C, t1[:],
                    op0=OP.mult, op1=OP.subtract)
                nc.scalar.activation(out=t1[:], in_=t2[:], func=AF.Sqrt,
                                     bias=eps1[:])
                with nc.allow_low_precision(reason="rstd in bf16 is intended"):
                    nc.vector.reciprocal(rstdbf[:, ts_], t1[:])
            mu_b = stat.tile([P, TKV], DT_BF, tag="mu_b")
            nc.gpsimd.partition_broadcast(mu_b[:, :ntok], mubf[:, :ntok])
            rstd_b = stat.tile([P, TKV], DT_BF, tag="rstd_b")
            nc.gpsimd.partition_broadcast(rstd_b[:, :ntok], rstdbf[:, :ntok])
            return mu_b[:, :ntok], rstd_b[:, :ntok]

        def ln_apply(dst, x_bf, mu_b, rstd_b, ntok):
            nc.vector.tensor_sub(
                dst[:], x_bf[:], mu_b[:, None, :].broadcast_to((P, CT, ntok)))
            nc.vector.tensor_mul(
                dst[:], dst[:], rstd_b[:, None, :].broadcast_to((P, CT, ntok)))

        # ---------- LN1 over all kv tokens (q tokens are cols [0,TQ)) ----
        mark("ln1kv")
        X = big.tile([P, CT, TKV], DT_BF, tag="X")
        for ct in range(CT):
            nc.sync.dma_start(out=X[:, ct, :], in_=d_xkv.ap()[ct])
        mu_b, rstd_b = ln_stats(X, TKV)
        xc = big.tile([P, CT, TKV], DT_F8, tag="xc")
        ln_apply(xc, X, mu_b, rstd_b, TKV)
        # X raw stays for the attention residual; pre-add the proj bias so
        # the proj copy-back is a single scalar_tensor_tensor.
        for ct in range(CT):
            nc.vector.tensor_scalar(
                out=X[:, ct, :TQ], in0=X[:, ct, :TQ],
                scalar1=bproj_sb[:, ct : ct + 1], scalar2=None, op0=OP.add)

        # ---------- Q projection (feature-major, fp8 DoubleRow) ----------
        mark("qgemm")
        Q = big.tile([P, CT, TQ], DT_BF, tag="Q")
        for qch in range(4):
            wch = wpool.tile([P, CT, 256], DT_F8, tag="w8")
            nc.sync.dma_start(
                out=wch[:],
                in_=d_wqkv.ap()[:, :, qch * 256 : (qch + 1) * 256].rearrange(
                    "c p f -> p c f"))
            for fsub in range(2):
                fo = qch * 2 + fsub
                for tt in range(TQ // 512):
                    ps = ps1.tile([P, 512], DT_F32, tag="g")
                    for cp in range(CT // 2):
                        nc.tensor.matmul(
                            ps[:],
                            wch[:, 2 * cp : 2 * cp + 2, fsub * P : (fsub + 1) * P],
                            xc[:, 2 * cp : 2 * cp + 2, tt * 512 : (tt + 1) * 512],
                            start=(cp == 0), stop=(cp == CT // 2 - 1),
                            perf_mode=PM.DoubleRow)
                    nc.vector.tensor_scalar(
                        out=Q[:, fo, tt * 512 : (tt + 1) * 512],
                        in0=ps[:], scalar1=1.0 / SQ,
                        scalar2=bq_sb[:, fo : fo + 1],
                        op0=OP.mult, op1=OP.add)

        Y = big.tile([P, CT, TQ], DT_F8, tag="Y")

        # ---------- attention, K/V streamed per head pair ----------
        for hp in range(CT):
            mark(f"attn{hp}" if hp else "attn0_k")
            # K for this pair: [128ch, TKV] fm bf16 (scores stay bf16)
            wk = kvw.tile([P, CT, P], DT_F8, tag="wk")
            nc.sync.dma_start(
                out=wk[:],
                in_=d_wqkv.ap()[:, :, C + hp * P : C + (hp + 1) * P].rearrange(
                    "c p f -> p c f"))
            K_hp = ktp.tile([P, TKV], DT_BF, tag="kt")
            for tt in range(TKV // 512):
                ts_ = slice(tt * 512, (tt + 1) * 512)
                ps = ps1.tile([P, 512], DT_F32, tag="g")
                for cp in range(CT // 2):
                    nc.tensor.matmul(
                        ps[:], wk[:, 2 * cp : 2 * cp + 2, :],
                        xc[:, 2 * cp : 2 * cp + 2, ts_],
                        start=(cp == 0), stop=(cp == CT // 2 - 1),
                        perf_mode=PM.DoubleRow)
                nc.vector.tensor_scalar(
                    out=K_hp[:, ts_], in0=ps[:], scalar1=1.0 / SW,
                    scalar2=bk_sb[:, hp : hp + 1], op0=OP.mult, op1=OP.add)

            if hp == 0:
                mark("attn0_v")
            # V for this pair: token-major [tk, 2, hd+1] fp8 with ones col
            wv = kvw.tile([P, CT, P], DT_F8, tag="wv")
            nc.sync.dma_start(
                out=wv[:],
                in_=d_wqkv.ap()[:, :, 2 * C + hp * P : 2 * C + (hp + 1) * P
                                ].rearrange("c p f -> p c f"))
            V_hp = vtp.tile([P, 2, TKV // (2 * P), 2, HD + 2], DT_F8, tag="vt")
            nc.vector.memset(V_hp[:, :, :, :, HD : HD + 2], 1.0)
            for tk in range(TKV // P):
                ps = ps1.tile([P, 512], DT_F32, tag="g")
                for cp in range(CT // 2):
                    nc.tensor.matmul(
                        ps[:, :P],
                        xc[:, 2 * cp : 2 * cp + 2, tk * P : (tk + 1) * P],
                        wv[:, 2 * cp : 2 * cp + 2, :],
                        start=(cp == 0), stop=(cp == CT // 2 - 1),
                        perf_mode=PM.DoubleRow)
                nc.vector.scalar_tensor_tensor(
                    V_hp[:, tk % 2, tk // 2, :, 0:HD],
                    ps[:, :P].rearrange("p (h d) -> p h d", h=2), 1.0 / SW,
                    bv_b[:, 2 * hp : 2 * hp + 2, :],
                    op0=OP.mult, op1=OP.add)

            if hp == 0:
                mark("attn0_sc")
            for tcn in range(TQ // 512):
                tqs = slice(tcn * 512, (tcn + 1) * 512)
                pts = [ppool.tile([P, TKV // P, 512], DT_F8, tag="pt8",
                                  name=f"pt{i}") for i in range(2)]
                for g in range(TKV // 256):
                    psc = [ps2.tile([P, 1024], DT_F32, tag="sc",
                                    name=f"sc{i}") for i in range(2)]
                    for k2 in range(2):
                        tk = g * 2 + k2
                        for hi in range(2):
                            bp = hi * 64
                            nc.tensor.matmul(
                                psc[hi][:, k2 * 512 : (k2 + 1) * 512],
                                K_hp[bp : bp + 64, tk * P : (tk + 1) * P],
                                Q[bp : bp + 64, hp, tqs],
                                start=True, stop=True,
                                tile_position=(bp, 0))
                    for hi in range(2):
                        nc.scalar.activation(
                            out=pts[hi][:, g * 2 : g * 2 + 2, :],
                            in_=psc[hi][:].rearrange("p (k t) -> p k t", k=2),
                            func=AF.Exp)
                if hp == 0:
                    mark(f"attn0_av{tcn}")
                for hi in range(2):
                    ps_y = ps1.tile([P, 512], DT_F32, tag="g")
                    for g in range(TKV // 256):
                        nc.tensor.matmul(
                            ps_y[0 : HD + 2, :],
                            V_hp[:, :, g, hi, :],
                            pts[hi][:, 2 * g : 2 * g + 2, :],
                            start=(g == 0), stop=(g == TKV // 256 - 1),
                            perf_mode=PM.DoubleRow)
                    rrow = small.tile([1, 512], DT_F32, tag="rrow")
                    nc.vector.reciprocal(rrow[:], ps_y[HD : HD + 1, :])
                    rb = small.tile([HD, 512], DT_F32, tag="rb")
                    nc.gpsimd.partition_broadcast(rb[:], rrow[:])
                    if hi == 0:
                        nc.vector.tensor_mul(
                            Y[0:HD, hp, tqs], ps_y[0:HD, :], rb[:])
                    else:
                        # DVE lanes are partition-locked; odd head's rows
                        # must move to partitions 64-127 via DMA.
                        ytmp = small.tile([HD, 512], DT_F8, tag="ytmp")
                        nc.vector.tensor_mul(ytmp[:], ps_y[0:HD, :], rb[:])
                        nc.sync.dma_start(out=Y[HD:P, hp, tqs], in_=ytmp[:])

        # ---------- proj+residual -> LN2 (both chunks), then MLP ----------
        mark("proj")
        x2 = big.tile([P, CT, TQ], DT_BF, tag="x2")
        xc2 = big.tile([P, CT, TQ], DT_BF, tag="Q")   # reuse Q slot
        for tcn in range(TQ // 512):
            tqs = slice(tcn * 512, (tcn + 1) * 512)
            if tcn:
                mark(f"proj{tcn}")
            for pch in range(4):
                wch = wpool.tile([P, CT, 256], DT_F8, tag="w8")
                nc.sync.dma_start(
                    out=wch[:],
                    in_=d_wproj.ap()[:, :, pch * 256 : (pch + 1) * 256].rearrange(
                        "c p f -> p c f"))
                for fsub in range(2):
                    co = pch * 2 + fsub
                    ps = ps1.tile([P, 512], DT_F32, tag="g")
                    for cp in range(CT // 2):
                        nc.tensor.matmul(
                            ps[:],
                            wch[:, 2 * cp : 2 * cp + 2, fsub * P : (fsub + 1) * P],
                            Y[:, 2 * cp : 2 * cp + 2, tqs],
                            start=(cp == 0), stop=(cp == CT // 2 - 1),
                            perf_mode=PM.DoubleRow)
                    # x2 = ps/SW + (Xq + bproj)   (bias pre-added into X)
                    nc.vector.scalar_tensor_tensor(
                        x2[:, co, tqs], ps[:], 1.0 / SW,
                        X[:, co, tqs], op0=OP.mult, op1=OP.add)

            # LN2 for this chunk
            mark(f"ln2_{tcn}")
            mu2_b, rstd2_b = ln_stats(x2[:, :, tqs], 512)
            ln_apply(xc2[:, :, tqs], x2[:, :, tqs], mu2_b, rstd2_b, 512)

        for tcn in range(TQ // 512):
            tqs = slice(tcn * 512, (tcn + 1) * 512)
            # MLP fc + gelu (bf16)
            mark(f"mlp{tcn}")
            h3 = [ppool.tile([P, FT // 2, 512], DT_BF, tag="pt",
                             name=f"h3_{i}") for i in range(2)]
            for fch in range(F // 256):
                wch = wpool.tile([P, CT, 256], DT_BF, tag="w4")
                nc.sync.dma_start(
                    out=wch[:],
                    in_=d_wfc.ap()[:, :, fch * 256 : (fch + 1) * 256].rearrange(
                        "c p f -> p c f"))
                for fsub in range(2):
                    fo = fch * 2 + fsub
                    ps = ps1.tile([P, 512], DT_F32, tag="g")
                    for ci in range(CT):
                        nc.tensor.matmul(
                            ps[:],
                            wch[:, ci, fsub * P : (fsub + 1) * P],
                            xc2[:, ci, tqs],
                            start=(ci == 0), stop=(ci == CT - 1))
                    nc.scalar.activation(
                        out=h3[fo // 16][:, fo % 16, :], in_=ps[:],
                        func=AF.Gelu_apprx_tanh,
                        bias=bfc_sb[:, fo : fo + 1])
            for co in range(CT):
                wc2a = wpool.tile([P, FT // 2, P], DT_BF, tag="w4")
                nc.sync.dma_start(out=wc2a[:], in_=d_wfc2.ap()[co][:, 0 : FT // 2, :])
                wc2b = wpool.tile([P, FT // 2, P], DT_BF, tag="w4")
                nc.sync.dma_start(out=wc2b[:], in_=d_wfc2.ap()[co][:, FT // 2 :, :])
                wparts = (wc2a, wc2b)
                ps = ps1.tile([P, 512], DT_F32, tag="g")
                for fk in range(FT):
                    nc.tensor.matmul(
                        ps[:],
                        wparts[fk // 16][:, fk % 16, :],
                        h3[fk // 16][:, fk % 16, :],
                        start=(fk == 0), stop=(fk == FT - 1))
                osb = opool.tile([P, 512], DT_F32, tag="osb")
                nc.vector.scalar_tensor_tensor(
                    osb[:], ps[:], bfc2_sb[:, co : co + 1], x2[:, co, tqs],
                    op0=OP.add, op1=OP.add)
                nc.sync.dma_start(out=d_out.ap()[co][:, tqs], in_=osb[:])

        loop_ctx.__exit__(None, None, None)

        for pm in reversed(pools):
            pm.__exit__(None, None, None)

    nc.compile()
    return nc


def _get_nc():
    if "nc" not in _CACHE:
        _CACHE["nc"] = _build_nc()
    return _CACHE["nc"]


def _prep_shared(inputs):
    f32 = np.float32
    ln1_w = np.asarray(inputs["ln1_w"], f32)
    ln1_b = np.asarray(inputs["ln1_b"], f32)
    attn_w = np.asarray(inputs["attn_w"], f32)
    attn_b = np.asarray(inputs["attn_b"], f32)
    proj_w = np.asarray(inputs["proj_w"], f32)
    proj_b = np.asarray(inputs["proj_b"], f32)
    ln2_w = np.asarray(inputs["ln2_w"], f32)
    ln2_b = np.asarray(inputs["ln2_b"], f32)
    fc_w = np.asarray(inputs["fc_w"], f32)
    fc_b = np.asarray(inputs["fc_b"], f32)
    fc2_w = np.asarray(inputs["fc2_w"], f32)
    fc2_b = np.asarray(inputs["fc2_b"], f32)

    w1 = (ln1_w[:, None] * attn_w).copy()
    b1 = (ln1_b @ attn_w + attn_b).copy()
    w1[:, :C] *= 0.125 * SQ     # fold 1/sqrt(hd) + fp8 scale into Wq
    b1[:C] *= 0.125             # bias applied after the 1/SQ unscale
    w1[:, C:] *= SW             # fp8 scale for Wk/Wv
    w2 = ln2_w[:, None] * fc_w
    b2 = ln2_b @ fc_w + fc_b

    return {
        "wqkv": np.ascontiguousarray(w1.reshape(CT, P, 3 * C)).astype(_F8),
        "bq": np.ascontiguousarray(b1[:C].reshape(CT, P).T).astype(f32),
        "bk": np.ascontiguousarray(b1[C : 2 * C].reshape(CT, P).T).astype(f32),
        "bv": b1[2 * C :].reshape(1, C).astype(_BF16),
        "wproj": np.ascontiguousarray((proj_w * SW).reshape(CT, P, C)).astype(_F8),
        "bproj": np.ascontiguousarray(proj_b.reshape(CT, P).T).astype(f32),
        "wfc": np.ascontiguousarray(w2.reshape(CT, P, F)).astype(_BF16),
        "bfc": np.ascontiguousarray(b2.reshape(FT, P).T).astype(f32),
        "wfc2": np.ascontiguousarray(
            fc2_w.reshape(FT, P, CT, P).transpose(2, 1, 0, 3)).astype(_BF16),
        "bfc2": np.ascontiguousarray(fc2_b.reshape(CT, P).T).astype(f32),
    }


def _make_in_maps(inputs):
    x = np.asarray(inputs["x"], np.float32)  # [B, T, C]
    shared = _prep_shared(inputs)
    in_maps = []
    for core in range(NCORES):
        b, h = core // 2, core % 2
        # permute kv tokens: this core's q half first
        xp = np.concatenate(
            [x[b, h * TQ : (h + 1) * TQ], x[b, (1 - h) * TQ : (2 - h) * TQ]], 0)
        xT = np.ascontiguousarray(xp.T)                             # [C, TKV]
        m = dict(shared)
        m["xkv_bf"] = xT.reshape(CT, P, TKV).astype(_BF16)
        in_maps.append(m)
    return in_maps


def kernel(**inputs) -> np.ndarray:
    from concourse.bass_utils import run_bass_kernel_spmd

    nc = _get_nc()
    in_maps = _make_in_maps(inputs)
    res = run_bass_kernel_spmd(nc, in_maps, core_ids=list(range(NCORES)))

    out = np.empty((4, 2048, C), np.float32)
    for core in range(NCORES):
        b, h = core // 2, core % 2
        o = np.asarray(res.results[core]["out"])  # [CT, P, TQ]
        out[b, h * TQ : (h + 1) * TQ, :] = o.reshape(C, TQ).T
    return out


# revision 6
# speedup vs baseline: 3.2771x; 1.5371x over previous
"""Trainium2 Bass kernel for one GPT-2-style transformer Block.

Reference math: non-causal MHA + tanh-GELU MLP, both pre-LayerNorm with
residual. B=4, T=2048, C=1024, H=16 heads, hd=64.

Strategy: zero-communication data parallelism over 8 NeuronCores.
Core i handles batch b=i//2 and query-token half h=i%2 (1024 tokens).
Each core redundantly computes K,V for its batch's full 2048 tokens
(cheaper than a 2-rank collective), then attention/proj/MLP for its own
1024 query tokens only. The kv token order is host-permuted so the
core's own query tokens are always columns [0, 1024) — softmax over kv
is order-invariant, and this lets every core address its q slice at a
fixed offset (no per-core xq copy, single LN1 pass).

Device layout: activations feature-major [channels, tokens] ("fm"); V is
produced token-major for the attention AV matmul. Host pre-work is pure
layout/algebra: transpose x, fold LN affine into the next matmul
(LN_aff(xhat)@W + b == xhat@(ln_w*W) + (ln_b@W + b)), fold 1/sqrt(hd)
into Wq/bq.

fp8: QKV/V/AV/proj GEMMs run float8e4 with MatmulPerfMode.DoubleRow
(two 128-deep k-subtiles per instruction -> 2x PE throughput). Weights
are host-scaled by SW=32 (SQ=256 for the 0.125-folded Wq) into fp8
range; the inverse scale rides the PSUM->SBUF bias-add for free.
LN1 output xc is written fp8; exp output P^T is written fp8 by the
activation op; Y is fp8 into the proj GEMM. Scores stay bf16 (the
64-deep per-head contraction cannot use DoubleRow without waste) and
the MLP stays bf16 (fp8 there costs ~1.6e-2 max-rel: too close to the
2e-2 gate).

Attention: scores transposed S^T[tk,tq] (two heads of a pair live on
partitions 0-63 / 64-127 and their K=64 matmuls run concurrently via
row-group tiling); exp on ACT into fp8 P^T; AV uses V augmented with
ones columns (M=66, parity-blocked [P,2,8,2,66] so the DoubleRow
k-subtile stride meets the 16-element ISA alignment rule) and PSUM row
64 accumulates the softmax denominator for free; y is normalized with
the broadcast reciprocal on copy-back.
LayerNorm channel-stats are ones-vector matmuls on the TensorEngine.
K and V are computed per-head-pair inside the attention loop so their
GEMMs overlap the previous pair's softmax exp on ACT.
"""

import sys

import numpy as np
import ml_dtypes

if "/opt/trn_rl_repo" not in sys.path:
    sys.path.insert(0, "/opt/trn_rl_repo")

P = 128
C = 1024
CT = C // P            # 8 channel tiles
TKV = 2048
TQ = 1024
H = 16
HD = 64
F = 4096
FT = F // P            # 32
NCORES = 8
EPS = 1e-5
SW = 32.0              # fp8 weight scale (K/V/proj)
SQ = 256.0             # fp8 weight scale for Wq (0.125 pre-folded)

_BF16 = ml_dtypes.bfloat16
_F8 = ml_dtypes.float8_e4m3
_CACHE: dict = {}
PHASE_MARKS: list = []


def _build_nc(loop_n: int = 0):
    import concourse.tile as tile
    from concourse import bacc, mybir

    DT_BF = mybir.dt.bfloat16
    DT_F8 = mybir.dt.float8e4
    DT_F32 = mybir.dt.float32
    AF = mybir.ActivationFunctionType
    OP = mybir.AluOpType
    PM = mybir.MatmulPerfMode

    nc = bacc.Bacc("TRN2", target_bir_lowering=False)

    d_xkv = nc.declare_dram_parameter("xkv_bf", [CT, P, TKV], DT_BF, isOutput=False)
    d_wqkv = nc.declare_dram_parameter("wqkv", [CT, P, 3 * C], DT_F8, isOutput=False)
    d_bq = nc.declare_dram_parameter("bq", [P, CT], DT_F32, isOutput=False)
    d_bk = nc.declare_dram_parameter("bk", [P, CT], DT_F32, isOutput=False)
    d_bv = nc.declare_dram_parameter("bv", [1, C], DT_BF, isOutput=False)
    d_wproj = nc.declare_dram_parameter("wproj", [CT, P, C], DT_F8, isOutput=False)
    d_bproj = nc.declare_dram_parameter("bproj", [P, CT], DT_F32, isOutput=False)
    d_wfc = nc.declare_dram_parameter("wfc", [CT, P, F], DT_BF, isOutput=False)
    d_bfc = nc.declare_dram_parameter("bfc", [P, FT], DT_F32, isOutput=False)
    d_wfc2 = nc.declare_dram_parameter("wfc2", [CT, P, FT, P], DT_BF, isOutput=False)
    d_bfc2 = nc.declare_dram_parameter("bfc2", [P, CT], DT_F32, isOutput=False)
    d_out = nc.declare_dram_parameter("out", [CT, P, TQ], DT_F32, isOutput=True)

    import contextlib

    with tile.TileContext(nc) as tc:
        loop_ctx = tc.For_i(0, loop_n, 1) if loop_n else contextlib.nullcontext()
        pools = []

        def pool(name, bufs, space="SBUF"):
            pm = tc.tile_pool(name=name, bufs=bufs, space=space)
            pools.append(pm)
            return pm.__enter__()

        singles = pool("singles", 1)
        big = pool("big", 1)          # persistent tensors, explicit tag reuse
        stat = pool("stat", 1)        # LN stat rows (slot-shared across LNs)
        tmp = pool("tmp", 2)          # x^2 chunks
        small = pool("small", 2)      # reciprocal rows / broadcasts / ytmp
        wpool = pool("wpool", 3)      # streamed weight chunks
        kvw = pool("kvw", 2)          # small per-head-pair weight chunks
        ktp = pool("ktp", 2)          # per-head-pair K tiles
        vtp = pool("vtp", 2)          # per-head-pair V tiles
        ppool = pool("ppool", 2)      # P^T tiles (fp8) + h3 halves (bf16)
        opool = pool("opool", 1)
        ps1 = pool("ps1", 4, space="PSUM")   # 1-bank psums, tag "g"
        ps2 = pool("ps2", 2, space="PSUM")   # 2-bank score psums, tag "sc"

        loop_ctx.__enter__()

        PHASE_MARKS.clear()

        def mark(name):
            PHASE_MARKS.append((name, nc.next_id()))

        mark("setup")
        # constants / biases
        ones_bf = singles.tile([P, 1], DT_BF)
        nc.vector.memset(ones_bf, 1.0)
        eps1 = singles.tile([1, 1], DT_F32)
        nc.vector.memset(eps1, EPS)
        bq_sb = singles.tile([P, CT], DT_F32)
        nc.sync.dma_start(out=bq_sb, in_=d_bq[:, :])
        bk_sb = singles.tile([P, CT], DT_F32)
        nc.sync.dma_start(out=bk_sb, in_=d_bk[:, :])
        bproj_sb = singles.tile([P, CT], DT_F32)
        nc.sync.dma_start(out=bproj_sb, in_=d_bproj[:, :])
        bfc_sb = singles.tile([P, FT], DT_F32)
        nc.sync.dma_start(out=bfc_sb, in_=d_bfc[:, :])
        bfc2_sb = singles.tile([P, CT], DT_F32)
        nc.sync.dma_start(out=bfc2_sb, in_=d_bfc2[:, :])
        bv_row = singles.tile([1, C], DT_BF)
        nc.sync.dma_start(out=bv_row, in_=d_bv[:, :])
        bv_b = singles.tile([P, H, HD], DT_BF)
        nc.gpsimd.partition_broadcast(bv_b[:], bv_row[:])

        def ln_pass(dst, x_bf, ntok):
            """Fused LN over [P, CT, ntok] bf16 fm -> dst (any dtype).
            Per-512-chunk stats -> broadcast -> apply, so chunk t+1's
            stats overlap chunk t's apply and downstream GEMMs can start
            on early chunks. The apply multiply runs on GPSIMD (idle
            during this phase) so DVE only carries the subtract."""
            mubf = stat.tile([1, TKV], DT_BF, tag="mubf")
            rstdbf = stat.tile([1, TKV], DT_BF, tag="rstdbf")
            mu_b = stat.tile([P, TKV], DT_BF, tag="mu_b")
            rstd_b = stat.tile([P, TKV], DT_BF, tag="rstd_b")
            for tt in range(ntok // 512):
                ts_ = slice(tt * 512, (tt + 1) * 512)
                ps_s = ps1.tile([1, 512], DT_F32, tag="g")
                ps_q = ps1.tile([1, 512], DT_F32, tag="g")
                for ct in range(CT):
                    x2c = tmp.tile([P, 512], DT_BF, tag="x2c")
                    nc.gpsimd.tensor_mul(x2c[:], x_bf[:, ct, ts_], x_bf[:, ct, ts_])
                    nc.tensor.matmul(
                        ps_s[:], ones_bf[:], x_bf[:, ct, ts_],
                        start=(ct == 0), stop=(ct == CT - 1))
                    nc.tensor.matmul(
                        ps_q[:], ones_bf[:], x2c[:],
                        start=(ct == 0), stop=(ct == CT - 1))
                nc.vector.tensor_scalar_mul(mubf[:, ts_], ps_s[:], 1.0 / C)
                t1 = stat.tile([1, 512], DT_F32, tag="t1")
                nc.vector.tensor_mul(t1[:], mubf[:, ts_], mubf[:, ts_])
                t2 = stat.tile([1, 512], DT_F32, tag="t2")
                nc.vector.scalar_tensor_tensor(
                    t2[:], ps_q[:], 1.0 / C, t1[:],
                    op0=OP.mult, op1=OP.subtract)
                nc.scalar.activation(out=t1[:], in_=t2[:], func=AF.Sqrt,
                                     bias=eps1[:])
                with nc.allow_low_precision(reason="rstd in bf16 is intended"):
                    nc.vector.reciprocal(rstdbf[:, ts_], t1[:])
                nc.gpsimd.partition_broadcast(mu_b[:, ts_], mubf[:, ts_])
                nc.gpsimd.partition_broadcast(rstd_b[:, ts_], rstdbf[:, ts_])
                nc.vector.tensor_sub(
                    dst[:, :, ts_], x_bf[:, :, ts_],
                    mu_b[:, None, ts_].broadcast_to((P, CT, 512)))
                nc.gpsimd.tensor_mul(
                    dst[:, :, ts_], dst[:, :, ts_],
                    rstd_b[:, None, ts_].broadcast_to((P, CT, 512)))

        # ---------- LN1 over all kv tokens (q tokens are cols [0,TQ)) ----
        mark("ln1kv")
        X = big.tile([P, CT, TKV], DT_BF, tag="X")
        for ct in range(CT):
            nc.sync.dma_start(out=X[:, ct, :], in_=d_xkv.ap()[ct])
        xc = big.tile([P, CT, TKV], DT_F8, tag="xc")
        ln_pass(xc, X, TKV)
        # X raw stays for the attention residual; pre-add the proj bias so
        # the proj copy-back is a single scalar_tensor_tensor.
        for ct in range(CT):
            nc.vector.tensor_scalar(
                out=X[:, ct, :TQ], in0=X[:, ct, :TQ],
                scalar1=bproj_sb[:, ct : ct + 1], scalar2=None, op0=OP.add)

        # ---------- Q projection (feature-major, fp8 DoubleRow) ----------
        mark("qgemm")
        Q = big.tile([P, CT, TQ], DT_BF, tag="Q")
        for qch in range(4):
            wch = wpool.tile([P, CT, 256], DT_F8, tag="w8")
            nc.sync.dma_start(
                out=wch[:],
                in_=d_wqkv.ap()[:, :, qch * 256 : (qch + 1) * 256].rearrange(
                    "c p f -> p c f"))
            for fsub in range(2):
                fo = qch * 2 + fsub
                for tt in range(TQ // 512):
                    ps = ps1.tile([P, 512], DT_F32, tag="g")
                    for cp in range(CT // 2):
                        nc.tensor.matmul(
                            ps[:],
                            wch[:, 2 * cp : 2 * cp + 2, fsub * P : (fsub + 1) * P],
                            xc[:, 2 * cp : 2 * cp + 2, tt * 512 : (tt + 1) * 512],
                            start=(cp == 0), stop=(cp == CT // 2 - 1),
                            perf_mode=PM.DoubleRow)
                    nc.vector.tensor_scalar(
                        out=Q[:, fo, tt * 512 : (tt + 1) * 512],
                        in0=ps[:], scalar1=1.0 / SQ,
                        scalar2=bq_sb[:, fo : fo + 1],
                        op0=OP.mult, op1=OP.add)

        Y = big.tile([P, CT, TQ], DT_F8, tag="Y")

        # ---------- attention, K/V streamed per head pair ----------
        for hp in range(CT):
            mark(f"attn{hp}" if hp else "attn0_k")
            # K for this pair: [128ch, TKV] fm bf16 (scores stay bf16)
            wk = kvw.tile([P, CT, P], DT_F8, tag="wk")
            nc.sync.dma_start(
                out=wk[:],
                in_=d_wqkv.ap()[:, :, C + hp * P : C + (hp + 1) * P].rearrange(
                    "c p f -> p c f"))
            K_hp = ktp.tile([P, TKV], DT_BF, tag="kt")
            for tt in range(TKV // 512):
                ts_ = slice(tt * 512, (tt + 1) * 512)
                ps = ps1.tile([P, 512], DT_F32, tag="g")
                for cp in range(CT // 2):
                    nc.tensor.matmul(
                        ps[:], wk[:, 2 * cp : 2 * cp + 2, :],
                        xc[:, 2 * cp : 2 * cp + 2, ts_],
                        start=(cp == 0), stop=(cp == CT // 2 - 1),
                        perf_mode=PM.DoubleRow)
                nc.vector.tensor_scalar(
                    out=K_hp[:, ts_], in0=ps[:], scalar1=1.0 / SW,
                    scalar2=bk_sb[:, hp : hp + 1], op0=OP.mult, op1=OP.add)

            if hp == 0:
                mark("attn0_v")
            # V for this pair: token-major [tk, 2, hd+1] fp8 with ones col
            wv = kvw.tile([P, CT, P], DT_F8, tag="wv")
            nc.sync.dma_start(
                out=wv[:],
                in_=d_wqkv.ap()[:, :, 2 * C + hp * P : 2 * C + (hp + 1) * P
                                ].rearrange("c p f -> p c f"))
            V_hp = vtp.tile([P, 2, TKV // (2 * P), 2, HD + 2], DT_F8, tag="vt")
            nc.vector.memset(V_hp[:, :, :, :, HD : HD + 2], 1.0)
            for tk in range(TKV // P):
                ps = ps1.tile([P, 512], DT_F32, tag="g")
                for cp in range(CT // 2):
                    nc.tensor.matmul(
                        ps[:, :P],
                        xc[:, 2 * cp : 2 * cp + 2, tk * P : (tk + 1) * P],
                        wv[:, 2 * cp : 2 * cp + 2, :],
                        start=(cp == 0), stop=(cp == CT // 2 - 1),
                        perf_mode=PM.DoubleRow)
                nc.vector.scalar_tensor_tensor(
                    V_hp[:, tk % 2, tk // 2, :, 0:HD],
                    ps[:, :P].rearrange("p (h d) -> p h d", h=2), 1.0 / SW,
                    bv_b[:, 2 * hp : 2 * hp + 2, :],
                    op0=OP.mult, op1=OP.add)

            if hp == 0:
                mark("attn0_sc")
            for tcn in range(TQ // 512):
                tqs = slice(tcn * 512, (tcn + 1) * 512)
                pts = [ppool.tile([P, TKV // P, 512], DT_F8, tag="pt8",
                                  name=f"pt{i}") for i in range(2)]
                for g in range(TKV // 256):
                    psc = [ps2.tile([P, 1024], DT_F32, tag="sc",
                                    name=f"sc{i}") for i in range(2)]
                    for k2 in range(2):
                        tk = g * 2 + k2
                        for hi in range(2):
                            bp = hi * 64
                            nc.tensor.matmul(
                                psc[hi][:, k2 * 512 : (k2 + 1) * 512],
                                K_hp[bp : bp + 64, tk * P : (tk + 1) * P],
                                Q[bp : bp + 64, hp, tqs],
                                start=True, stop=True,
                                tile_position=(bp, 0))
                    for hi in range(2):
                        nc.scalar.activation(
                            out=pts[hi][:, g * 2 : g * 2 + 2, :],
                            in_=psc[hi][:].rearrange("p (k t) -> p k t", k=2),
                            func=AF.Exp)
                if hp == 0:
                    mark(f"attn0_av{tcn}")
                for hi in range(2):
                    ps_y = ps1.tile([P, 512], DT_F32, tag="g")
                    for g in range(TKV // 256):
                        nc.tensor.matmul(
                            ps_y[0 : HD + 2, :],
                            V_hp[:, :, g, hi, :],
                            pts[hi][:, 2 * g : 2 * g + 2, :],
                            start=(g == 0), stop=(g == TKV // 256 - 1),
                            perf_mode=PM.DoubleRow)
                    rrow = small.tile([1, 512], DT_F32, tag="rrow")
                    nc.vector.reciprocal(rrow[:], ps_y[HD : HD + 1, :])
                    rb = small.tile([HD, 512], DT_F32, tag="rb")
                    nc.gpsimd.partition_broadcast(rb[:], rrow[:])
                    if hi == 0:
                        nc.vector.tensor_mul(
                            Y[0:HD, hp, tqs], ps_y[0:HD, :], rb[:])
                    else:
                        # DVE lanes are partition-locked; odd head's rows
                        # must move to partitions 64-127 via DMA.
                        ytmp = small.tile([HD, 512], DT_F8, tag="ytmp")
                        nc.vector.tensor_mul(ytmp[:], ps_y[0:HD, :], rb[:])
                        nc.sync.dma_start(out=Y[HD:P, hp, tqs], in_=ytmp[:])

        # ---------- proj+residual -> LN2 (both chunks), then MLP ----------
        mark("proj")
        x2 = big.tile([P, CT, TQ], DT_BF, tag="x2")
        xc2 = big.tile([P, CT, TQ], DT_BF, tag="Q")   # reuse Q slot
        for tcn in range(TQ // 512):
            tqs = slice(tcn * 512, (tcn + 1) * 512)
            if tcn:
                mark(f"proj{tcn}")
            for pch in range(4):
                wch = wpool.tile([P, CT, 256], DT_F8, tag="w8")
                nc.sync.dma_start(
                    out=wch[:],
                    in_=d_wproj.ap()[:, :, pch * 256 : (pch + 1) * 256].rearrange(
                        "c p f -> p c f"))
                for fsub in range(2):
                    co = pch * 2 + fsub
                    ps = ps1.tile([P, 512], DT_F32, tag="g")
                    for cp in range(CT // 2):
                        nc.tensor.matmul(
                            ps[:],
                            wch[:, 2 * cp : 2 * cp + 2, fsub * P : (fsub + 1) * P],
                            Y[:, 2 * cp : 2 * cp + 2, tqs],
                            start=(cp == 0), stop=(cp == CT // 2 - 1),
                            perf_mode=PM.DoubleRow)
                    # x2 = ps/SW + (Xq + bproj)   (bias pre-added into X)
                    nc.vector.scalar_tensor_tensor(
                        x2[:, co, tqs], ps[:], 1.0 / SW,
                        X[:, co, tqs], op0=OP.mult, op1=OP.add)

            # LN2 for this chunk
            mark(f"ln2_{tcn}")
            ln_pass(xc2[:, :, tqs], x2[:, :, tqs], 512)

        for tcn in range(TQ // 512):
            tqs = slice(tcn * 512, (tcn + 1) * 512)
            # MLP fc + gelu (bf16)
            mark(f"mlp{tcn}")
            h3 = [ppool.tile([P, FT // 2, 512], DT_BF, tag="pt",
                             name=f"h3_{i}") for i in range(2)]
            for fch in range(F // 256):
                wch = wpool.tile([P, CT, 256], DT_BF, tag="w4")
                nc.sync.dma_start(
                    out=wch[:],
                    in_=d_wfc.ap()[:, :, fch * 256 : (fch + 1) * 256].rearrange(
                        "c p f -> p c f"))
                for fsub in range(2):
                    fo = fch * 2 + fsub
                    ps = ps1.tile([P, 512], DT_F32, tag="g")
                    for ci in range(CT):
                        nc.tensor.matmul(
                            ps[:],
                            wch[:, ci, fsub * P : (fsub + 1) * P],
                            xc2[:, ci, tqs],
                            start=(ci == 0), stop=(ci == CT - 1))
                    nc.scalar.activation(
                        out=h3[fo // 16][:, fo % 16, :], in_=ps[:],
                        func=AF.Gelu_apprx_tanh,
                        bias=bfc_sb[:, fo : fo + 1])
            for co in range(CT):
                wc2a = wpool.tile([P, FT // 2, P], DT_BF, tag="w4")
                nc.sync.dma_start(out=wc2a[:], in_=d_wfc2.ap()[co][:, 0 : FT // 2, :])
                wc2b = wpool.tile([P, FT // 2, P], DT_BF, tag="w4")
                nc.sync.dma_start(out=wc2b[:], in_=d_wfc2.ap()[co][:, FT // 2 :, :])
                wparts = (wc2a, wc2b)
                ps = ps1.tile([P, 512], DT_F32, tag="g")
                for fk in range(FT):
                    nc.tensor.matmul(
                        ps[:],
                        wparts[fk // 16][:, fk % 16, :],
                        h3[fk // 16][:, fk % 16, :],
                        start=(fk == 0), stop=(fk == FT - 1))
                osb = opool.tile([P, 512], DT_F32, tag="osb")
                nc.vector.scalar_tensor_tensor(
                    osb[:], ps[:], bfc2_sb[:, co : co + 1], x2[:, co, tqs],
                    op0=OP.add, op1=OP.add)
                nc.sync.dma_start(out=d_out.ap()[co][:, tqs], in_=osb[:])

        loop_ctx.__exit__(None, None, None)

        for pm in reversed(pools):
            pm.__exit__(None, None, None)

    nc.compile()
    return nc


def _get_nc():
    if "nc" not in _CACHE:
        _CACHE["nc"] = _build_nc()
    return _CACHE["nc"]


def _prep_shared(inputs):
    f32 = np.float32
    ln1_w = np.asarray(inputs["ln1_w"], f32)
    ln1_b = np.asarray(inputs["ln1_b"], f32)
    attn_w = np.asarray(inputs["attn_w"], f32)
    attn_b = np.asarray(inputs["attn_b"], f32)
    proj_w = np.asarray(inputs["proj_w"], f32)
    proj_b = np.asarray(inputs["proj_b"], f32)
    ln2_w = np.asarray(inputs["ln2_w"], f32)
    ln2_b = np.asarray(inputs["ln2_b"], f32)
    fc_w = np.asarray(inputs["fc_w"], f32)
    fc_b = np.asarray(inputs["fc_b"], f32)
    fc2_w = np.asarray(inputs["fc2_w"], f32)
    fc2_b = np.asarray(inputs["fc2_b"], f32)

    w1 = (ln1_w[:, None] * attn_w).copy()
    b1 = (ln1_b @ attn_w + attn_b).copy()
    w1[:, :C] *= 0.125 * SQ     # fold 1/sqrt(hd) + fp8 scale into Wq
    b1[:C] *= 0.125             # bias applied after the 1/SQ unscale
    w1[:, C:] *= SW             # fp8 scale for Wk/Wv
    w2 = ln2_w[:, None] * fc_w
    b2 = ln2_b @ fc_w + fc_b

    return {
        "wqkv": np.ascontiguousarray(w1.reshape(CT, P, 3 * C)).astype(_F8),
        "bq": np.ascontiguousarray(b1[:C].reshape(CT, P).T).astype(f32),
        "bk": np.ascontiguousarray(b1[C : 2 * C].reshape(CT, P).T).astype(f32),
        "bv": b1[2 * C :].reshape(1, C).astype(_BF16),
        "wproj": np.ascontiguousarray((proj_w * SW).reshape(CT, P, C)).astype(_F8),
        "bproj": np.ascontiguousarray(proj_b.reshape(CT, P).T).astype(f32),
        "wfc": np.ascontiguousarray(w2.reshape(CT, P, F)).astype(_BF16),
        "bfc": np.ascontiguousarray(b2.reshape(FT, P).T).astype(f32),
        "wfc2": np.ascontiguousarray(
            fc2_w.reshape(FT, P, CT, P).transpose(2, 1, 0, 3)).astype(_BF16),
        "bfc2": np.ascontiguousarray(fc2_b.reshape(CT, P).T).astype(f32),
    }


def _make_in_maps(inputs):
    x = np.asarray(inputs["x"], np.float32)  # [B, T, C]
    shared = _prep_shared(inputs)
    in_maps = []
    for core in range(NCORES):
        b, h = core // 2, core % 2
        # permute kv tokens: this core's q half first
        xp = np.concatenate(
            [x[b, h * TQ : (h + 1) * TQ], x[b, (1 - h) * TQ : (2 - h) * TQ]], 0)
        xT = np.ascontiguousarray(xp.T)                             # [C, TKV]
        m = dict(shared)
        m["xkv_bf"] = xT.reshape(CT, P, TKV).astype(_BF16)
        in_maps.append(m)
    return in_maps


def kernel(**inputs) -> np.ndarray:
    from concourse.bass_utils import run_bass_kernel_spmd

    nc = _get_nc()
    in_maps = _make_in_maps(inputs)
    res = run_bass_kernel_spmd(nc, in_maps, core_ids=list(range(NCORES)))

    out = np.empty((4, 2048, C), np.float32)
    for core in range(NCORES):
        b, h = core // 2, core % 2
        o = np.asarray(res.results[core]["out"])  # [CT, P, TQ]
        out[b, h * TQ : (h + 1) * TQ, :] = o.reshape(C, TQ).T
    return out
